# revision 44
# baseline (speedup 1.0000x reference)
"""GraphSAGE (2-layer, mean-agg) edge-scoring kernel for 8 trn2 NeuronCores.

v2 — overlap + compaction redesign:
  - Batch-parallel: core c handles edges [512c, 512(c+1)).
  - Tables COMPACTED per pair: only rows actually gathered by the pair's
    edges are projected (u: ~45k/50k, i: ~68k/100k). Rows renumbered in
    projection order so each core writes a contiguous block of the
    pair-shared table (4KB/partition descriptors via RPP row packing).
  - The two strict all-engine barriers are replaced by two async pair
    AllReduces (one per table) + explicit add_dep_helper gating: all
    user-table gathers/aggregations run DURING item-table projection.
  - Hop-2 segment aggregation software-pipelined into the projection
    emission order; SAGE hop-1 consumed in 640-col streaming blocks.
"""
import os
import numpy as np

F0 = F1 = 10
B = 4096
NCORES = 8
EDGES = B // NCORES          # 512
P = 128
D = 256
NU, NI = 50000, 100000
SEG_GROUPS = 512             # hop-2 groups per segment
NSEG = (EDGES * F0) // SEG_GROUPS    # 10
BLK = 640                    # h1 streaming block (64 groups)
PROJ_TILE = 1024
RPP = 8                      # table rows packed per partition in scatter
GCALL = 1024                 # max idx per dma_gather call


def _wrap16(a):
    a = np.asarray(a, np.int16)
    w = a.reshape(-1, 16).T
    return np.tile(w, (8, 1)).astype(np.int16)


def _pad128(n):
    return (n + 127) & ~127


def _nch_chu(T):
    nch = -(-T // 32000)
    chu = -(-T // (nch * 128)) * 128
    return nch, chu


class _HopPlan:
    def __init__(self, idx_lists, nrows, chu, out_order, fixed_plen=None):
        self.chu = chu
        self.nch = -(-nrows // chu)
        M = len(idx_lists[0])
        runs = []
        for A in idx_lists:
            ch = A // chu
            runs.append([np.where(ch == c)[0] for c in range(self.nch)])
        if fixed_plen is None:
            fixed_plen = [
                _pad128(max(len(r[c]) for r in runs)) for c in range(self.nch)]
        self.plen = fixed_plen
        self.runs = runs
        self.offs = np.concatenate([[0], np.cumsum(self.plen)]).astype(np.int64)
        self.tot = int(self.offs[-1])
        self.idx, self.rid = [], []
        for core, A in enumerate(idx_lists):
            iv = np.zeros(self.tot, np.int16)
            p2s = np.empty(M, np.int64)
            for c in range(self.nch):
                pos = runs[core][c]
                off = int(self.offs[c])
                iv[off:off + len(pos)] = (A[pos] - c * chu).astype(np.int16)
                p2s[pos] = off + np.arange(len(pos))
            self.idx.append(iv)
            self.rid.append(p2s[out_order].astype(np.int16))
        self.calls = []
        for c in range(self.nch):
            off, rem = int(self.offs[c]), self.plen[c]
            while rem > 0:
                n = min(GCALL, rem)
                self.calls.append((c, off, n))
                off += n
                rem -= n


class _GPlan:
    """Banded group-sum selection matrices, variable width per stage tile."""
    def __init__(self, plans, ngroups, fanout):
        nseg = len(plans)
        ntiles = plans[0].tot // P
        gmin = np.full((nseg, ntiles), 10**9, np.int64)
        gmax = np.full((nseg, ntiles), -1, np.int64)
        for s, pl in enumerate(plans):
            for core in range(NCORES):
                for c in range(pl.nch):
                    pos = pl.runs[core][c]
                    if len(pos) == 0:
                        continue
                    slots = pl.offs[c] + np.arange(len(pos))
                    grp = pos // fanout
                    t = slots // P
                    np.minimum.at(gmin, (s, t), grp)
                    np.maximum.at(gmax, (s, t), grp)
        nonempty = gmax >= 0
        wt = np.where(nonempty, ((gmax - gmin + 1) + 7) & ~7, 0)
        wt = np.minimum(wt, ngroups).astype(np.int64)
        c0 = np.minimum(np.where(nonempty, gmin, 0), ngroups - wt)
        c0 = np.maximum(c0, 0).astype(np.int64)
        off = np.zeros((nseg, ntiles), np.int64)
        segtot = np.zeros(nseg, np.int64)
        for s in range(nseg):
            off[s] = np.concatenate([[0], np.cumsum(wt[s])[:-1]])
            segtot[s] = wt[s].sum()
        self.wt, self.c0, self.off = wt, c0, off
        self.segtot = segtot
        self.segoff = np.concatenate([[0], np.cumsum(segtot)]).astype(np.int64)
        self.total = int(self.segoff[-1])
        G = np.zeros((NCORES, P, self.total), np.float16)
        for s, pl in enumerate(plans):
            for core in range(NCORES):
                for c in range(pl.nch):
                    pos = pl.runs[core][c]
                    if len(pos) == 0:
                        continue
                    slots = pl.offs[c] + np.arange(len(pos))
                    grp = pos // fanout
                    t = slots // P
                    p = slots % P
                    col = self.segoff[s] + off[s, t] + grp - c0[s, t]
                    G[core, p, col] = 1.0
        self.G = G


def _compact(inputs):
    """Per-pair table compaction + renumbering in projection order."""
    h = {}
    for k, n in (("src_h0", EDGES), ("src_h1", EDGES * F0),
                 ("src_h2", EDGES * F0 * F1), ("dst_h0", EDGES),
                 ("dst_h1", EDGES * F0), ("dst_h2", EDGES * F0 * F1)):
        h[k] = np.asarray(inputs[k]).astype(np.int64).reshape(NCORES, n)
    rows = {"u": [], "i": []}   # per pair: (rows_even, rows_odd)
    for pr in range(NCORES // 2):
        cs = (2 * pr, 2 * pr + 1)
        u_draws = np.concatenate([h["src_h0"][c] for c in cs]
                                 + [h["src_h2"][c] for c in cs]
                                 + [h["dst_h1"][c] for c in cs])
        i_draws = np.concatenate([h["dst_h0"][c] for c in cs]
                                 + [h["dst_h2"][c] for c in cs]
                                 + [h["src_h1"][c] for c in cs])
        uu = np.unique(u_draws)
        ii = np.unique(i_draws)
        rows["u"].append((uu[0::2], uu[1::2]))
        rows["i"].append((ii[0::2], ii[1::2]))
    nt = {}
    for t in ("u", "i"):
        mx = max(max(len(a), len(b)) for a, b in rows[t])
        nt[t] = -(-mx // PROJ_TILE)
    T = {t: 2 * nt[t] * PROJ_TILE for t in ("u", "i")}
    # new-id maps per pair
    newid = {"u": [], "i": []}
    for t, N in (("u", NU), ("i", NI)):
        for pr in range(NCORES // 2):
            m = np.full(N, -1, np.int64)
            ev, od = rows[t][pr]
            m[ev] = np.arange(len(ev))
            m[od] = nt[t] * PROJ_TILE + np.arange(len(od))
            newid[t].append(m)
    # remap hop indices (hop->table: src h0/h2 + dst h1 -> u; rest -> i)
    tbl = {"src_h0": "u", "src_h1": "i", "src_h2": "u",
           "dst_h0": "i", "dst_h1": "u", "dst_h2": "i"}
    hnew = {}
    for k, t in tbl.items():
        out = np.empty_like(h[k])
        for c in range(NCORES):
            out[c] = newid[t][c // 2][h[k][c]]
        assert (out >= 0).all()
        hnew[k] = out
    return hnew, rows, nt, T


def _build_plans(hnew, T):
    plans = {}
    chu = {t: _nch_chu(T[t])[1] for t in ("u", "i")}
    for side, (t0, t1, t2) in {"s": ("u", "i", "u"),
                               "d": ("i", "u", "i")}.items():
        pre = "src" if side == "s" else "dst"
        h0 = hnew[pre + "_h0"]
        h1 = hnew[pre + "_h1"]
        h2 = hnew[pre + "_h2"]
        plans[side + "0"] = _HopPlan([h0[c] for c in range(NCORES)], T[t0],
                                     chu[t0], np.arange(EDGES))
        p1 = _HopPlan([h1[c] for c in range(NCORES)], T[t1], chu[t1],
                      np.arange(EDGES * F0))
        plans[side + "1"] = p1
        plans[side + "1G"] = _GPlan([p1], EDGES, F0)
        nch2 = -(-T[t2] // chu[t2])
        seglists = [
            [h2[c][s * SEG_GROUPS * F1:(s + 1) * SEG_GROUPS * F1]
             for c in range(NCORES)] for s in range(NSEG)]
        plen = [0] * nch2
        for s in range(NSEG):
            for A in seglists[s]:
                ch = A // chu[t2]
                for c in range(nch2):
                    plen[c] = max(plen[c], _pad128(int((ch == c).sum())))
        seg2 = [_HopPlan(seglists[s], T[t2], chu[t2],
                         np.arange(SEG_GROUPS * F1), fixed_plen=plen)
                for s in range(NSEG)]
        plans[side + "2"] = seg2
        plans[side + "2G"] = _GPlan(seg2, SEG_GROUPS, F1)
    plans["chu"] = chu
    return plans


_XTPERM = (np.arange(PROJ_TILE) % P) * RPP + np.arange(PROJ_TILE) // P


def _xt_host(feat, row_list, ntiles, reverse=False):
    """Per-core transposed f16 features in block-permuted projection order."""
    L = ntiles * PROJ_TILE
    padded = np.full(L, -1, np.int64)
    padded[:len(row_list)] = row_list
    blocks = padded.reshape(ntiles, PROJ_TILE)
    if reverse:
        blocks = blocks[::-1]
    order = blocks[:, _XTPERM].reshape(-1)
    safe = np.where(order < 0, 0, order)
    x = feat[safe].astype(np.float16)
    x[order < 0] = 0
    return np.ascontiguousarray(x.T)


def _build_bass(plans, ntu, nti, T_u, T_i, zb=False, debug=False):
    import concourse.bass as bass
    import concourse.tile as tile
    import concourse.bacc as bacc
    from concourse import mybir, library_config
    from concourse.tile_rust import add_dep_helper
    from contextlib import ExitStack

    f16 = mybir.dt.float16
    f32 = mybir.dt.float32
    i16 = mybir.dt.int16
    i32 = mybir.dt.int32
    AF = mybir.ActivationFunctionType

    nc = bacc.Bacc("TRN2", target_bir_lowering=False, debug=False,
                   num_devices=NCORES, num_swdge_queues=4)

    chu = plans["chu"]
    xt_u = nc.dram_tensor("xt_u", [512, ntu * PROJ_TILE], f16,
                          kind="ExternalInput")
    xt_i = nc.dram_tensor("xt_i", [512, nti * PROJ_TILE], f16,
                          kind="ExternalInput")
    prow_u = nc.dram_tensor("prow_u", [P, ntu], i32, kind="ExternalInput")
    prow_i = nc.dram_tensor("prow_i", [P, nti], i32, kind="ExternalInput")
    w_pu = nc.dram_tensor("w_pu", [P, 4, D], f16, kind="ExternalInput")
    w_pi = nc.dram_tensor("w_pi", [P, 4, D], f16, kind="ExternalInput")
    b_bc = nc.dram_tensor("b_bc", [P, 2, D], f32, kind="ExternalInput")
    wsage = nc.dram_tensor("wsage", [P, 2, 2 * 768], f16, kind="ExternalInput")
    wlin = nc.dram_tensor("wlin", [P, 1], f16, kind="ExternalInput")
    blin = nc.dram_tensor("blin", [1, 1], f32, kind="ExternalInput")

    idx_t, rid_t, g_t = {}, {}, {}
    for sd in ("s", "d"):
        p0, p1, seg2 = plans[sd + "0"], plans[sd + "1"], plans[sd + "2"]
        gp1, gp2 = plans[sd + "1G"], plans[sd + "2G"]
        t2 = seg2[0].tot
        idx_t[sd + "0"] = nc.dram_tensor(f"idx{sd}0", [P, p0.tot // 16], i16,
                                         kind="ExternalInput")
        rid_t[sd + "0"] = nc.dram_tensor(f"rid{sd}0", [P, EDGES // 16], i16,
                                         kind="ExternalInput")
        idx_t[sd + "1"] = nc.dram_tensor(f"idx{sd}1", [P, p1.tot // 16], i16,
                                         kind="ExternalInput")
        rid_t[sd + "1"] = nc.dram_tensor(f"rid{sd}1", [P, EDGES * F0 // 16],
                                         i16, kind="ExternalInput")
        idx_t[sd + "2"] = nc.dram_tensor(f"idx{sd}2", [P, NSEG * t2 // 16],
                                         i16, kind="ExternalInput")
        g_t[sd + "1"] = nc.dram_tensor(f"g{sd}1", [P, gp1.total], f16,
                                       kind="ExternalInput")
        g_t[sd + "2"] = nc.dram_tensor(f"g{sd}2", [P, gp2.total], f16,
                                       kind="ExternalInput")

    out = nc.dram_tensor("out", [1, EDGES], f32, kind="ExternalOutput")
    dbg = (nc.dram_tensor("dbg", [P, 2, EDGES], f32, kind="ExternalOutput")
           if debug else None)

    tab_u = nc.dram_tensor("tab_u", [T_u, D], f16, addr_space="Shared")
    tab_i = nc.dram_tensor("tab_i", [T_i, D], f16, addr_space="Shared")
    tabs = {"u": tab_u, "i": tab_i}
    cc_in_u = nc.dram_tensor("cc_in_u", [1, 16], f32)
    cc_out_u = nc.dram_tensor("cc_out_u", [1, 16], f32)
    cc_in_ia = nc.dram_tensor("cc_in_ia", [1, 16], f32)
    cc_out_ia = nc.dram_tensor("cc_out_ia", [1, 16], f32)
    cc_in_i = nc.dram_tensor("cc_in_i", [1, 16], f32)
    cc_out_i = nc.dram_tensor("cc_out_i", [1, 16], f32)

    PAIRS = [[0, 1], [2, 3], [4, 5], [6, 7]]

    with tile.TileContext(nc) as tc, ExitStack() as ctx:
        nc.gpsimd.load_library(library_config.mlp)
        tc.strict_bb_all_engine_barrier()

        wpool = ctx.enter_context(tc.tile_pool(name="w", bufs=1))
        ppool = ctx.enter_context(tc.tile_pool(name="proj", bufs=2))
        pspool = ctx.enter_context(tc.tile_pool(name="pps", bufs=4,
                                                space="PSUM"))
        aggps = ctx.enter_context(tc.tile_pool(name="aggps", bufs=2,
                                               space="PSUM"))
        ipool = ctx.enter_context(tc.tile_pool(name="idx", bufs=1))
        spool = ctx.enter_context(tc.tile_pool(name="stage", bufs=2))
        s1pool = ctx.enter_context(tc.tile_pool(name="stage1", bufs=1))
        hpool = ctx.enter_context(tc.tile_pool(name="hts", bufs=1))
        kpool = ctx.enter_context(tc.tile_pool(name="kblk", bufs=2))
        i2pool = ctx.enter_context(tc.tile_pool(name="idx2", bufs=2))
        g2pool = ctx.enter_context(tc.tile_pool(name="gmat", bufs=2))
        g1pool = ctx.enter_context(tc.tile_pool(name="gmat1", bufs=1))
        g2max = max(int(plans[sd + "2G"].segtot.max()) for sd in ("s", "d"))
        g1max = max(plans[sd + "1G"].total for sd in ("s", "d"))
        vpool = ctx.enter_context(tc.tile_pool(name="vtmp", bufs=1))
        gpool = ctx.enter_context(tc.tile_pool(name="gts", bufs=1))

        w_pu_s = wpool.tile([P, 4, D], f16, tag="wpu")
        w_pi_s = wpool.tile([P, 4, D], f16, tag="wpi")
        wsage_s = wpool.tile([P, 2, 2 * 768], f16, tag="wsage")
        wlin_s = wpool.tile([P, 1], f16, tag="wlin")
        blin_s = wpool.tile([1, 1], f32, tag="blin")
        prow_u_s = wpool.tile([P, ntu], i32, tag="prowu")
        prow_i_s = wpool.tile([P, nti], i32, tag="prowi")
        loads = [(w_pu_s, w_pu), (w_pi_s, w_pi), (wsage_s, wsage),
                 (wlin_s, wlin), (blin_s, blin), (prow_u_s, prow_u),
                 (prow_i_s, prow_i)]
        if not zb:
            b_bc_s = wpool.tile([P, 2, D], f32, tag="bbc")
            loads.append((b_bc_s, b_bc))
        for dst_, src_ in loads:
            nc.sync.dma_start(dst_[:], src_[:])

        zz = wpool.tile([1, 16], f32, tag="zz")
        nc.vector.memset(zz[:], 1.0)
        nc.sync.dma_start(cc_in_u[:], zz[:])
        nc.sync.dma_start(cc_in_ia[:], zz[:])
        nc.sync.dma_start(cc_in_i[:], zz[:])

        # preload ALL index / rid tiles
        it = {}
        for sd in ("s", "d"):
            p0, p1 = plans[sd + "0"], plans[sd + "1"]
            for nm, tt, shape in (
                    ("i0", idx_t[sd + "0"], [P, p0.tot // 16]),
                    ("r0", rid_t[sd + "0"], [P, EDGES // 16]),
                    ("i1", idx_t[sd + "1"], [P, p1.tot // 16]),
                    ("r1", rid_t[sd + "1"], [P, EDGES * F0 // 16])):
                tl = ipool.tile(shape, i16, tag=f"{nm}{sd}")
                nc.sync.dma_start(tl[:], tt[:])
                it[nm + sd] = tl

        # PE warmup
        wu = wpool.tile([P, 256], f16, tag="warm")
        nc.vector.memset(wu[:], 0.0)
        psw = pspool.tile([P, D], f32, tag="pps")
        for i in range(16):
            nc.tensor.matmul(out=psw[:, :P], lhsT=wu[:, :P],
                             rhs=wu[:, P:256], start=(i == 0), stop=(i == 15))

        qn = [0]
        S = {"s": {}, "d": {}}

        # ---------------- helpers ----------------
        def project_tile(xt, prow_s, w_s, bcol, tab, t):
            xtt = ppool.tile([P, 4, PROJ_TILE], f16, tag="xtt")
            nc.sync.dma_start(
                out=xtt[:],
                in_=xt[:, t * PROJ_TILE:(t + 1) * PROJ_TILE].rearrange(
                    "(c p) n -> p c n", p=P))
            sig = ppool.tile([P, RPP, D], f16, tag="sig")
            for j in range(RPP):
                ps = pspool.tile([P, D], f32, tag="pps")
                if not zb:
                    nc.vector.tensor_copy(ps[:], b_bc_s[:, bcol, :])
                for c in range(4):
                    nc.tensor.matmul(
                        out=ps[:], lhsT=xtt[:, c, j * P:(j + 1) * P],
                        rhs=w_s[:, c, :], start=(zb and c == 0),
                        stop=(c == 3))
                nc.scalar.activation(out=sig[:, j, :], in_=ps[:],
                                     func=AF.Sigmoid)
            w = nc.gpsimd.indirect_dma_start(
                out=tab[:, :].rearrange("(q r) d -> q (r d)", r=RPP),
                out_offset=bass.IndirectOffsetOnAxis(
                    ap=prow_s[:, t:t + 1], axis=0),
                in_=sig[:].rearrange("p r d -> p (r d)"), in_offset=None)
            return w

        def gather_hbm(plan, idx_sb, tab, stage, gate, ioff=0):
            insts = []
            for (c, off, n) in plan.calls:
                a = c * plan.chu
                b = min(a + plan.chu, tab.shape[0])
                g = nc.gpsimd.dma_gather(
                    stage[:, off // P:(off + n) // P, :],
                    tab[a:b, :],
                    idx_sb[:, (ioff + off) // 16:(ioff + off + n) // 16],
                    n, n, D, queue_num=1 + qn[0] % 3)
                qn[0] += 1
                gt_ = gate.get(c) if isinstance(gate, dict) else gate
                if gt_ is not None:
                    add_dep_helper(g.ins, gt_.ins, sync=True,
                                   reason="gate gather on pair cc")
                insts.append(g)
            return insts

        def regather(stage, rid_ap, n_out, dstT):
            g = nc.gpsimd.dma_gather(
                dstT[:], stage[:], rid_ap, n_out, n_out, D, transpose=True,
                sbuf_tokens_per_rank=P, sbuf_free_dim_per_rank=D * 2,
                queue_num=1 + qn[0] % 3)
            qn[0] += 1
            return g

        def tile_order(plan, tkey):
            # i-table chunk 1 is gated on the late cc_i barrier; multiply
            # those stage tiles last so chunk-0/2 work fills the PE hole
            nt = plan.tot // P
            if tkey != "i":
                return range(nt)
            tc_ = [c for c, pl in enumerate(plan.plen)
                   for _ in range(pl // P)]
            return ([t for t in range(nt) if tc_[t] != 1]
                    + [t for t in range(nt) if tc_[t] == 1])

        def agg_matmul(stage, gt, gp, s, pslo, pshi, order=None):
            ntiles = stage.shape[1]
            nc.vector.memset(pslo[:], 0.0)
            nc.vector.memset(pshi[:], 0.0)
            for t in (order if order is not None else range(ntiles)):
                w = int(gp.wt[s, t])
                if w == 0:
                    continue
                a = int(gp.c0[s, t])
                o = int(gp.off[s, t])
                rhs = gt[:, o:o + w]
                nc.tensor.matmul(out=pslo[:, a:a + w],
                                 lhsT=stage[:, t, 0:P], rhs=rhs,
                                 start=False, stop=False)
                nc.tensor.matmul(out=pshi[:, a:a + w],
                                 lhsT=stage[:, t, P:D], rhs=rhs,
                                 start=False, stop=False)

        def tree10(src, dst, ngr):
            # src [P,2,ngr*10] f16 -> dst [P,2,ngr] (sum over last-dim 10s)
            t0_t = vpool.tile([P, 2, BLK // F0], f16, tag="tr0")
            t0 = t0_t[:, :, :ngr]
            t1_t = vpool.tile([P, 2, BLK // F0], f16, tag="tr1")
            t1 = t1_t[:, :, :ngr]
            v = src.rearrange("p c (j k) -> p c j k", k=F0)
            nc.vector.tensor_add(t0[:], v[:, :, :, 0], v[:, :, :, 1])
            for i in range(1, 5):
                nc.vector.tensor_add(t1[:], v[:, :, :, 2 * i],
                                     v[:, :, :, 2 * i + 1])
                if i < 4:
                    nc.vector.tensor_add(t0[:], t0[:], t1[:])
            nc.vector.tensor_add(dst, t0[:], t1[:])

        def side_w(si):
            wof = si * 768
            return (wsage_s[:, :, wof:wof + D],
                    wsage_s[:, :, wof + D:wof + 2 * D],
                    wsage_s[:, :, wof + 2 * D:wof + 2 * D + 128],
                    wsage_s[:, :, wof + 2 * D + 128:wof + 768])

        def do_h0(sd, tkey, gate):
            p0 = plans[sd + "0"]
            st0 = spool.tile([P, p0.tot // P, D], f16, tag="stg")
            gather_hbm(p0, it["i0" + sd], tabs[tkey], st0, gate)
            h0T = hpool.tile([P, 2, EDGES], f16, tag=f"h0T{sd}")
            regather(st0, it["r0" + sd][:, :], EDGES, h0T)
            S[sd]["h0T"] = h0T

        def do_h2_seg(sd, tkey, s, gate):
            seg2 = plans[sd + "2"]
            gp2 = plans[sd + "2G"]
            t2 = seg2[0].tot
            nt2 = t2 // P
            if s == 0:
                n1T_new = hpool.tile([P, 2, EDGES * F0], f16, tag=f"n1T{sd}")
                S[sd]["n1T"] = n1T_new
            n1T = S[sd]["n1T"]
            i2 = i2pool.tile([P, t2 // 16], i16, tag="i2")
            nc.sync.dma_start(i2[:], idx_t[sd + "2"][:, s * t2 // 16:
                                                     (s + 1) * t2 // 16])
            gt = g2pool.tile([P, g2max], f16, tag="gt")
            st_ = int(gp2.segtot[s])
            so = int(gp2.segoff[s])
            nc.sync.dma_start(gt[:, :st_], g_t[sd + "2"][:, so:so + st_])
            st2 = spool.tile([P, nt2, D], f16, tag="stg")
            gather_hbm(seg2[s], i2, tabs[tkey], st2, gate)
            pslo = aggps.tile([P, SEG_GROUPS], f32, tag="agglo")
            pshi = aggps.tile([P, SEG_GROUPS], f32, tag="agghi")
            agg_matmul(st2, gt, gp2, s, pslo, pshi,
                       order=tile_order(seg2[s], tkey))
            sl = slice(s * SEG_GROUPS, (s + 1) * SEG_GROUPS)
            nc.vector.tensor_copy(n1T[:, 0, sl], pslo[:])
            nc.vector.tensor_copy(n1T[:, 1, sl], pshi[:])

        def do_h1(sd, tkey, gate):
            p1 = plans[sd + "1"]
            gp1 = plans[sd + "1G"]
            nt1 = p1.tot // P
            g1sb = g1pool.tile([P, g1max], f16, tag="g1sb")
            nc.sync.dma_start(g1sb[:, :gp1.total], g_t[sd + "1"][:])
            st1 = s1pool.tile([P, nt1, D], f16, tag="stg1")
            gather_hbm(p1, it["i1" + sd], tabs[tkey], st1, gate)
            pslo = aggps.tile([P, EDGES], f32, tag="agglo")
            pshi = aggps.tile([P, EDGES], f32, tag="agghi")
            agg_matmul(st1, g1sb, gp1, 0, pslo, pshi,
                       order=tile_order(p1, tkey))
            n0T = hpool.tile([P, 2, EDGES], f16, tag=f"n0T{sd}")
            nc.vector.tensor_copy(n0T[:, 0, :], pslo[:])
            nc.vector.tensor_copy(n0T[:, 1, :], pshi[:])
            S[sd]["n0T"] = n0T
            S[sd]["st1"] = st1

        def do_h1_regather(sd):
            st1 = S[sd]["st1"]
            h1T = hpool.tile([P, 2, EDGES * F0], f16, tag=f"h1T{sd}")
            for b in range(EDGES * F0 // BLK):
                tmp = kpool.tile([P, 2, BLK], f16, tag="reT")
                regather(st1, it["r1" + sd][:, b * BLK // 16:
                                            (b + 1) * BLK // 16],
                         BLK, tmp)
                nc.vector.tensor_copy(h1T[:, :, b * BLK:(b + 1) * BLK],
                                      tmp[:])
            S[sd]["h1T"] = h1T

        def sage_block(sd, si, b, nT):
            """layer-0 g1 for cols [b*BLK,(b+1)*BLK) + tree -> nT block."""
            ws0, wa0, _, _ = side_w(si)
            n1T = S[sd]["n1T"]
            if "h1T" in S[sd]:
                src = S[sd]["h1T"][:, :, b * BLK:(b + 1) * BLK]
            else:
                tmp = kpool.tile([P, 2, BLK], f16, tag="reT")
                regather(S[sd]["st1"],
                         it["r1" + sd][:, b * BLK // 16:(b + 1) * BLK // 16],
                         BLK, tmp)
                src = tmp[:]
            g1blk = kpool.tile([P, 2, BLK], f16, tag="g1b", bufs=1)
            for o in range(2):
                for half in range(2):
                    lo = half * (BLK // 2)
                    sl = slice(b * BLK + lo, b * BLK + lo + BLK // 2)
                    ps = pspool.tile([P, BLK // 2], f32, tag="pps")
                    for c in range(2):
                        nc.tensor.matmul(
                            out=ps[:], lhsT=ws0[:, c, o * P:(o + 1) * P],
                            rhs=src[:, c, lo:lo + BLK // 2],
                            start=(c == 0), stop=False)
                        nc.tensor.matmul(
                            out=ps[:], lhsT=wa0[:, c, o * P:(o + 1) * P],
                            rhs=n1T[:, c, sl], start=False, stop=(c == 1))
                    nc.scalar.activation(out=g1blk[:, o, lo:lo + BLK // 2],
                                         in_=ps[:], func=AF.Relu)
            ngr = BLK // F0
            tree10(g1blk[:], nT[:, :, b * ngr:(b + 1) * ngr], ngr)

        hts = {}

        def sage_tail(sd, si, nT):
            ws0, wa0, ws1, wa1 = side_w(si)
            h0T, n0T = S[sd]["h0T"], S[sd]["n0T"]
            g0T = gpool.tile([P, 2, EDGES], f16, tag="g0T")
            for o in range(2):
                ps = pspool.tile([P, EDGES], f32, tag="pps")
                for c in range(2):
                    nc.tensor.matmul(out=ps[:],
                                     lhsT=ws0[:, c, o * P:(o + 1) * P],
                                     rhs=h0T[:, c, :], start=(c == 0),
                                     stop=False)
                    nc.tensor.matmul(out=ps[:],
                                     lhsT=wa0[:, c, o * P:(o + 1) * P],
                                     rhs=n0T[:, c, :], start=False,
                                     stop=(c == 1))
                nc.scalar.activation(out=g0T[:, o, :], in_=ps[:], func=AF.Relu)
            ps = pspool.tile([P, EDGES], f32, tag="pps")
            for c in range(2):
                nc.tensor.matmul(out=ps[:], lhsT=ws1[:, c, :],
                                 rhs=g0T[:, c, :], start=(c == 0), stop=False)
                nc.tensor.matmul(out=ps[:], lhsT=wa1[:, c, :],
                                 rhs=nT[:, c, :], start=False, stop=(c == 1))
            hT = gpool.tile([P, EDGES], f16, tag=f"hT{sd}")
            nc.scalar.activation(out=hT[:], in_=ps[:], func=AF.Copy)
            hts[sd] = hT

        # ---------------- phase A: project u ----------------
        su = [project_tile(xt_u, prow_u_s, w_pu_s, 0, tab_u, t)
              for t in range(ntu)]
        cc_u = nc.gpsimd.collective_compute(
            "AllReduce", mybir.AluOpType.add, replica_groups=PAIRS,
            ins=[cc_in_u.ap()], outs=[cc_out_u.ap()])
        for w in su:
            add_dep_helper(cc_u.ins, w.ins, sync=True, reason="cc_u scatters")

        # ---------- project i, with phase-B (u-dependent) interleaved ------
        si_insts = []
        hooks = {}

        def add_hook(t, fn):
            hooks.setdefault(t, []).append(fn)

        add_hook(6, lambda: do_h0("s", "u", cc_u))
        add_hook(7, lambda: do_h1("d", "u", cc_u))
        for s in range(NSEG):
            add_hook(8 + (s * 27) // NSEG,
                     lambda s=s: do_h2_seg("s", "u", s, cc_u))
        # early barrier once both cores have written i-table chunk 0
        ia_tile = -(-chu["i"] // PROJ_TILE) - 1
        cc_ia_h = []

        def emit_cc_ia():
            cc = nc.gpsimd.collective_compute(
                "AllReduce", mybir.AluOpType.add, replica_groups=PAIRS,
                ins=[cc_in_ia.ap()], outs=[cc_out_ia.ap()])
            for w in si_insts[:ia_tile + 1]:
                add_dep_helper(cc.ins, w.ins, sync=True,
                               reason="cc_ia chunk0 scatters")
            add_dep_helper(cc.ins, cc_u.ins, sync=False, reason="cc order")
            cc_ia_h.append(cc)

        add_hook(ia_tile, emit_cc_ia)
        for t in range(nti):
            si_insts.append(project_tile(xt_i, prow_i_s, w_pi_s, 1, tab_i, t))
            for fn in hooks.get(t, []):
                fn()
        for t in sorted(k for k in hooks if k >= nti):
            for fn in hooks[t]:
                fn()
        cc_ia = cc_ia_h[0]
        cc_i = nc.gpsimd.collective_compute(
            "AllReduce", mybir.AluOpType.add, replica_groups=PAIRS,
            ins=[cc_in_i.ap()], outs=[cc_out_i.ap()])
        for w in si_insts:
            add_dep_helper(cc_i.ins, w.ins, sync=True, reason="cc_i scatters")
        add_dep_helper(cc_i.ins, cc_ia.ins, sync=False, reason="cc order")

        # ---------------- phase C': i-dependent ----------------
        # chunks 0 and 2 are complete at cc_ia (odd cores write reversed)
        gates_i = {0: cc_ia, 1: cc_i, 2: cc_ia}
        do_h1_regather("d")
        do_h0("d", "i", gates_i)
        do_h1("s", "i", gates_i)
        nT_s = gpool.tile([P, 2, EDGES], f16, tag="nTs")
        nT_d = gpool.tile([P, 2, EDGES], f16, tag="nTd")
        NB = EDGES * F0 // BLK
        bd = 0
        for s in range(NSEG):
            do_h2_seg("d", "i", s, gates_i)
            if s < NB:
                sage_block("s", 0, s, nT_s)
            while bd < NB and BLK * (bd + 1) <= SEG_GROUPS * (s + 1):
                sage_block("d", 1, bd, nT_d)
                bd += 1
        sage_tail("s", 0, nT_s)
        while bd < NB:
            sage_block("d", 1, bd, nT_d)
            bd += 1
        sage_tail("d", 1, nT_d)

        prod = vpool.tile([P, EDGES], f16, tag="prod")
        nc.vector.tensor_mul(prod[:], hts["s"][:], hts["d"][:])
        psf = pspool.tile([1, EDGES], f32, tag="pps")
        nc.tensor.matmul(out=psf[:], lhsT=wlin_s[:], rhs=prod[:],
                         start=True, stop=True)
        res = vpool.tile([1, EDGES], f32, tag="res")
        nc.scalar.activation(out=res[:], in_=psf[:], func=AF.Identity,
                             bias=blin_s[:, :1])
        nc.sync.dma_start(out[:], res[:])
        if debug:
            dv = gpool.tile([P, 2, EDGES], f32, tag="dv")
            nc.vector.tensor_copy(dv[:, 0, :], hts["s"][:])
            nc.vector.tensor_copy(dv[:, 1, :], hts["d"][:])
            nc.sync.dma_start(dbg[:], dv[:])

    nc.compile()
    return nc


def kernel(**inputs) -> np.ndarray:
    hnew, rows, nt, T = _compact(inputs)
    plans = _build_plans(hnew, T)
    ntu, nti = nt["u"], nt["i"]

    trace = bool(os.environ.get("GNN_TRACE"))
    debug = bool(os.environ.get("GNN_DEBUG"))
    if trace:
        try:
            import timing_shim
            timing_shim.install()
        except ImportError:
            trace = False
    from concourse.bass_utils import run_bass_kernel_spmd

    zb = (not np.any(np.asarray(inputs["b_pu"]))
          and not np.any(np.asarray(inputs["b_pi"])))
    nc = _build_bass(plans, ntu, nti, T["u"], T["i"], zb=zb, debug=debug)

    uf = np.asarray(inputs["user_feat"], np.float32)
    itf = np.asarray(inputs["item_feat"], np.float32)

    f16 = np.float16
    w_pu = np.ascontiguousarray(
        np.asarray(inputs["W_pu"], np.float32).reshape(4, P, D)
        .transpose(1, 0, 2)).astype(f16)
    w_pi = np.ascontiguousarray(
        np.asarray(inputs["W_pi"], np.float32).reshape(4, P, D)
        .transpose(1, 0, 2)).astype(f16)
    b_bc = np.ascontiguousarray(np.broadcast_to(
        np.stack([np.asarray(inputs["b_pu"], np.float32),
                  np.asarray(inputs["b_pi"], np.float32)])[None],
        (P, 2, D))).astype(np.float32)

    def sagew(pre):
        s0 = np.asarray(inputs[f"{pre}_self0"], np.float32)
        a0 = np.asarray(inputs[f"{pre}_agg0"], np.float32) * (1.0 / F0)
        s1 = np.asarray(inputs[f"{pre}_self1"], np.float32)
        a1 = np.asarray(inputs[f"{pre}_agg1"], np.float32) * (1.0 / F0)
        cat = np.concatenate([s0, a0, s1, a1], axis=1)  # [256, 768]
        return cat.reshape(2, P, 768).transpose(1, 0, 2)

    wsage = np.ascontiguousarray(
        np.concatenate([sagew("u"), sagew("i")], axis=2)).astype(f16)
    wlin = np.asarray(inputs["W_lin"], np.float32).astype(f16)
    blin = np.asarray(inputs["b_lin"], np.float32).reshape(1, 1)

    def mk_prow(ntl, par, reverse):
        base = par * ntl * PROJ_TILE // RPP
        blocks = np.arange(ntl)
        if reverse:
            blocks = blocks[::-1]
        return np.ascontiguousarray(
            (base + blocks[None, :] * P
             + np.arange(P)[:, None]).astype(np.int32))

    in_maps = []
    for c in range(NCORES):
        pr_, par = c // 2, c % 2
        # odd cores write the i-table tiles top-down so that chunk 2 (the
        # odd block's tail) is complete by the early cc_ia barrier
        m = {
            "xt_u": _xt_host(uf, rows["u"][pr_][par], ntu),
            "xt_i": _xt_host(itf, rows["i"][pr_][par], nti,
                             reverse=(par == 1)),
            "prow_u": mk_prow(ntu, par, False),
            "prow_i": mk_prow(nti, par, par == 1),
            "w_pu": w_pu, "w_pi": w_pi, "b_bc": b_bc,
            "wsage": wsage, "wlin": wlin, "blin": blin,
        }
        for sd in ("s", "d"):
            p0, p1, seg2 = plans[sd + "0"], plans[sd + "1"], plans[sd + "2"]
            m[f"idx{sd}0"] = _wrap16(p0.idx[c])
            m[f"rid{sd}0"] = _wrap16(p0.rid[c])
            m[f"idx{sd}1"] = _wrap16(p1.idx[c])
            m[f"rid{sd}1"] = _wrap16(p1.rid[c])
            m[f"idx{sd}2"] = np.concatenate(
                [_wrap16(pl.idx[c]) for pl in seg2], axis=1)
            m[f"g{sd}1"] = plans[sd + "1G"].G[c]
            m[f"g{sd}2"] = plans[sd + "2G"].G[c]
        in_maps.append(m)

    tcores = (list(range(NCORES)) if os.environ.get("GNN_TRACE_ALL")
              else [0])
    kw = dict(trace=True, trace_cores=tcores) if trace else {}
    res = run_bass_kernel_spmd(nc, in_maps, core_ids=list(range(NCORES)), **kw)
    if trace and res.exec_time_ns:
        print(f"HW exec time: {res.exec_time_ns} ns")
        kernel.last_exec_ns = res.exec_time_ns
    if debug:
        kernel.last_dbg = [res.results[c]["dbg"] for c in range(NCORES)]

    logits = np.concatenate([res.results[c]["out"][0] for c in range(NCORES)])
    return logits.reshape(B, 1).astype(np.float32)
